# revision 8
# baseline (speedup 1.0000x reference)
"""Trainium2 Bass kernel for 2D erosion (3x3 sliding-window min) on
x: (8, 4, 1024, 1024) f32.

Strategy: pure data parallel over the 32 (b, c) images -> 4 images per core.
All device traffic is bf16, which halves both DMA volume and DVE cycle
count (packed 2-byte ops run in the DVE 2x perf mode). bf16 keeps the max
rel err at ~4e-3 (well under the 2e-2 budget) for ALL magnitudes; fp16 was
rejected because its subnormal range (|x| < 6e-5) quantizes with up to
~1.5e-2 rel err against the harness denominator floor.

Per image, one SBUF tile [128 partitions x 8192]: partition p holds image
rows 8p..8p+7 concatenated along the free dim.
  - Vertical 3-tap min: pair-sharing s[k]=min(x[2k],x[2k+1]) then combines
    (DVE, all packed views). Partition-boundary rows come from a [128, 2W]
    halo tile filled by two on-chip SBUF->SBUF shift DMAs (no HBM re-reads);
    the image top/bottom border rows are a +big constant memset once into
    the persistent halo ring buffers.
  - Horizontal 3-tap min: shift-by-1 formulation t[j]=min(v[j],v[j+1]),
    out[j]=min(t[j-1],t[j]) keeps every AP packed (DVE 2x); the two per-row
    edge columns are fixed with tiny ACT copies. (The Pool engine rejects
    tensor_tensor on this compiler build, so all combines live on DVE.)
Engine budget per image ~= DVE 15us / DMA-bus 13.1us; slightly DVE-bound.
DMA queues: loads + halo shifts on SP, stores on ACT.
The program's first image splits its load (DVE starts ~3us earlier) and the
last image splits combine+store (final store overlaps compute) to trim the
pipeline fill/drain of a 1-rep run; steady-state is unchanged.
"""

import ml_dtypes
import numpy as np

import concourse.bass as bass
import concourse.bacc as bacc
import concourse.mybir as mybir
from concourse.tile import TileContext
from concourse.bass_utils import run_bass_kernel_spmd

N_CORES = 8
B, C, H, W = 8, 4, 1024, 1024
IMGS = B * C // N_CORES  # images per core = 4
P = 128                  # SBUF partitions
R = H // P               # image rows per partition = 8
F = R * W                # free-dim elements per partition = 8192
BIG = 1.0e9              # +inf stand-in (matches reference PAD)
BF16 = mybir.dt.bfloat16
MIN = mybir.AluOpType.min

_NC_CACHE = {}


def _build_nc(reps=1):
    nc = bacc.Bacc()
    x = nc.dram_tensor("x", (IMGS * H, W), BF16, kind="ExternalInput")
    y = nc.dram_tensor("y", (IMGS * H, W), BF16, kind="ExternalOutput")
    seq = [im for _ in range(reps) for im in range(IMGS)]
    last = len(seq) - 1

    with TileContext(nc) as tc:
        with (
            tc.tile_pool(name="xp", bufs=3) as xpool,
            tc.tile_pool(name="hp", bufs=2) as hpool,
            tc.tile_pool(name="sp", bufs=2) as spool,
            tc.tile_pool(name="vp", bufs=2) as vpool,
            tc.tile_pool(name="tp", bufs=2) as tpool,
        ):
            # Persistent halo ring: cols 0:W hold row 8p-1 (lo), W:2W hold
            # row 8p+8 (hi). Whole-tile memset (engine APs can't start at
            # partition 127); the per-image shift DMAs never write lo@p=0 /
            # hi@p=127, so those rows keep BIG for the image borders.
            halos = [
                hpool.tile([P, 2 * W], BF16, name=f"halo{j}") for j in range(2)
            ]
            for h in halos:
                nc.gpsimd.memset(h, BIG)

            for n, i in enumerate(seq):
                base = i * H
                first = n == 0

                xt = xpool.tile([P, F], BF16)
                xr = xt.rearrange("p (r w) -> p r w", r=R)
                if first:
                    # split load: the vertical pass's first op can start
                    # after half the rows have landed
                    xmr = x[base : base + H, :].rearrange("(p r) w -> p r w", p=P)
                    nc.sync.dma_start(out=xr[:, 0 : R // 2, :], in_=xmr[:, 0 : R // 2, :])
                    nc.sync.dma_start(out=xr[:, R // 2 : R, :], in_=xmr[:, R // 2 : R, :])
                else:
                    xm = x[base : base + H, :].rearrange("(p r) w -> p (r w)", p=P)
                    nc.sync.dma_start(out=xt, in_=xm)

                # on-chip partition shifts for the vertical-boundary rows
                halo = halos[n % 2]
                nc.sync.dma_start(
                    out=halo[1:P, 0:W], in_=xt[0 : P - 1, (R - 1) * W : R * W]
                )
                nc.sync.dma_start(out=halo[0 : P - 1, W : 2 * W], in_=xt[1:P, 0:W])

                s = spool.tile([P, (R // 2) * W], BF16)        # [128, 4096]
                sr = s.rearrange("p (r w) -> p r w", r=R // 2)
                v = vpool.tile([P, F], BF16)                   # vertical-min result
                vr = v.rearrange("p (r w) -> p r w", r=R)

                # ---- vertical pass (DVE): v[r] = min(x[r-1], x[r], x[r+1]) ----
                if first:
                    nc.vector.tensor_tensor(
                        out=sr[:, 0 : R // 4, :],
                        in0=xr[:, 0 : R // 2 : 2, :],
                        in1=xr[:, 1 : R // 2 : 2, :],
                        op=MIN,
                    )
                    nc.vector.tensor_tensor(
                        out=sr[:, R // 4 : R // 2, :],
                        in0=xr[:, R // 2 : R : 2, :],
                        in1=xr[:, R // 2 + 1 : R : 2, :],
                        op=MIN,
                    )
                else:
                    nc.vector.tensor_tensor(
                        out=sr, in0=xr[:, 0:R:2, :], in1=xr[:, 1:R:2, :], op=MIN
                    )
                nc.vector.tensor_tensor(
                    out=vr[:, 2:R:2, :],
                    in0=xr[:, 1 : R - 1 : 2, :],
                    in1=sr[:, 1 : R // 2, :],
                    op=MIN,
                )
                nc.vector.tensor_tensor(
                    out=vr[:, 1 : R - 1 : 2, :],
                    in0=sr[:, 0 : R // 2 - 1, :],
                    in1=xr[:, 2:R:2, :],
                    op=MIN,
                )
                # boundary rows {0, R-1}: min(halo, {s0, s3}) in one op
                nc.vector.tensor_tensor(
                    out=vr[:, 0 : R : R - 1, :],
                    in0=halo.rearrange("p (s w) -> p s w", s=2),
                    in1=sr[:, 0 : R // 2 : R // 2 - 1, :],
                    op=MIN,
                )

                # ---- horizontal pass: o[j] = min(v[j-1], v[j], v[j+1]) ----
                t = tpool.tile([P, F], BF16)
                nc.vector.tensor_tensor(
                    out=t[:, 0 : F - 1], in0=v[:, 0 : F - 1], in1=v[:, 1:F], op=MIN
                )
                xtr = xt.rearrange("p (r w) -> p r w", r=R)
                tr = t.rearrange("p (r w) -> p r w", r=R)
                ymr = y[base : base + H, :].rearrange("(p r) w -> p r w", p=P)
                if n == last:
                    # split combine / edge-fix / store by half so the first
                    # store overlaps the second combine (drain shortening)
                    HF = F // 2
                    HR = R // 2
                    nc.vector.tensor_tensor(
                        out=xt[:, 1:HF], in0=t[:, 0 : HF - 1], in1=t[:, 1:HF], op=MIN
                    )
                    nc.scalar.copy(out=xtr[:, 0:HR, 0:1], in_=tr[:, 0:HR, 0:1])
                    nc.scalar.copy(
                        out=xtr[:, 0:HR, W - 1 : W], in_=tr[:, 0:HR, W - 2 : W - 1]
                    )
                    nc.scalar.dma_start(out=ymr[:, 0:HR, :], in_=xtr[:, 0:HR, :])
                    nc.vector.tensor_tensor(
                        out=xt[:, HF : F - 1],
                        in0=t[:, HF - 1 : F - 2],
                        in1=t[:, HF : F - 1],
                        op=MIN,
                    )
                    nc.scalar.copy(out=xtr[:, HR:R, 0:1], in_=tr[:, HR:R, 0:1])
                    nc.scalar.copy(
                        out=xtr[:, HR:R, W - 1 : W], in_=tr[:, HR:R, W - 2 : W - 1]
                    )
                    nc.scalar.dma_start(out=ymr[:, HR:R, :], in_=xtr[:, HR:R, :])
                else:
                    # interior columns (row-crossing values fixed below)
                    nc.vector.tensor_tensor(
                        out=xt[:, 1 : F - 1],
                        in0=t[:, 0 : F - 2],
                        in1=t[:, 1 : F - 1],
                        op=MIN,
                    )
                    # per-row first/last column: window shrinks to 2 taps
                    nc.scalar.copy(out=xtr[:, :, 0:1], in_=tr[:, :, 0:1])
                    nc.scalar.copy(
                        out=xtr[:, :, W - 1 : W], in_=tr[:, :, W - 2 : W - 1]
                    )
                    # store on the ACT HWDGE ring (parallel to SP loads)
                    ym = y[base : base + H, :].rearrange("(p r) w -> p (r w)", p=P)
                    nc.scalar.dma_start(out=ym, in_=xt)

    nc.finalize()
    return nc


def _get_nc(reps=1):
    if reps not in _NC_CACHE:
        _NC_CACHE[reps] = _build_nc(reps)
    return _NC_CACHE[reps]


def kernel(x: np.ndarray, _reps: int = 1):
    x = np.asarray(x)
    assert x.shape == (B, C, H, W)
    x16 = np.asarray(x, dtype=ml_dtypes.bfloat16)
    xs = x16.reshape(N_CORES, IMGS * H, W)
    nc = _get_nc(_reps)
    in_maps = [{"x": xs[k]} for k in range(N_CORES)]
    res = run_bass_kernel_spmd(nc, in_maps, core_ids=list(range(N_CORES)))
    out = np.stack([r["y"] for r in res.results], axis=0).astype(np.float32)
    return out.reshape(B, C, H, W)
